# revision 3
# baseline (speedup 1.0000x reference)
"""GAT influence layer on 8 Trainium2 NeuronCores (Bass/Tile) — v2.3.

Strategy (node-permuted blocks, identity-stationary segment-sum):
  Pass 1 (device): Wh = h @ W (bf16 in, f32 psum), s_src/s_dst via an
      augmented weight matrix, node-sharded across cores.  Outputs Wh
      bf16 + s f32.
  Host: degree-sort nodes; 128-node blocks; block b -> (core b%8,
      row j=b//8); node at partition p = slot%128.  Edge (row,col) ->
      (core, p, tile column base[j]+rank).  Streams: msg = Wh[col] bf16,
      qs/qd fp16.  Pure gather/permutation.
  Pass 2 (device): exp(leakyrelu(qs+qd)) -> bf16 (+ pair-duplicated
      copy so the scaling tensor_tensor gets an innermost step-1 AP);
      X = msg * exp (DVE/GPSIMD split); segment sum via PSUM-accumulated
      [128,64] matmuls with a CONSTANT identity stationary; per-block
      denominator = reduce_sum(exp) scheduled per chunk; epilogue
      Copy*(1/den) on ScalarE, bf16 out.  The reference's global
      max-subtract cancels in the softmax ratio.
  Host: unpermute rows.
"""

import os
import numpy as np
import ml_dtypes

N_NODES = 100000
N_EDGES = 1600000
IN_DIM = 128
OUT_DIM = 64
NEG_SLOPE = 0.2
CORES = 8
NBPC = 98                     # 128-node blocks per core
NPP = NBPC * 128              # padded nodes per core (12544)
PAD_Q = -30000.0
CHUNK_TILES = 176             # target tiles per pass-2 pipeline chunk
DVE_NUM = 6                   # DVE takes DVE_NUM/DVE_DEN of scaling work
DVE_DEN = 7

LAST_STATS = {}


def _build_pass1():
    from concourse import bacc, mybir
    import concourse.tile as tile

    f32 = mybir.dt.float32
    bf16 = mybir.dt.bfloat16
    act = mybir.ActivationFunctionType
    nc = bacc.Bacc("TRN2", target_bir_lowering=False, debug=False)
    d_hT = nc.dram_tensor("hT", [128, NPP], bf16, kind="ExternalInput")
    d_W = nc.dram_tensor("Wm", [IN_DIM, OUT_DIM], bf16, kind="ExternalInput")
    d_WT = nc.dram_tensor("WT", [OUT_DIM, IN_DIM], bf16, kind="ExternalInput")
    d_a2 = nc.dram_tensor("a2", [OUT_DIM, 2], bf16, kind="ExternalInput")
    d_whT = nc.dram_tensor("whT", [OUT_DIM, NPP], bf16, kind="ExternalOutput")
    d_sT = nc.dram_tensor("sT", [2, NPP], f32, kind="ExternalOutput")

    NW = 512
    with tile.TileContext(nc) as tc:
        with tc.tile_pool(name="c1", bufs=1) as cp, \
             tc.tile_pool(name="ht1", bufs=6) as hp, \
             tc.tile_pool(name="wo1", bufs=4) as wo, \
             tc.tile_pool(name="psw", bufs=1, space="PSUM") as psw, \
             tc.tile_pool(name="ps1", bufs=6, space="PSUM") as psp:
            w_sb = cp.tile([IN_DIM, OUT_DIM], bf16)
            nc.sync.dma_start(out=w_sb[:], in_=d_W[:])
            wt_sb = cp.tile([OUT_DIM, IN_DIM], bf16)
            nc.sync.dma_start(out=wt_sb[:], in_=d_WT[:])
            a_sb = cp.tile([OUT_DIM, 2], bf16)
            nc.sync.dma_start(out=a_sb[:], in_=d_a2[:])

            waug = cp.tile([IN_DIM, OUT_DIM + 2], bf16)
            nc.vector.tensor_copy(out=waug[:, 0:OUT_DIM], in_=w_sb[:])
            ws_ps = psw.tile([IN_DIM, 2], f32, space="PSUM")
            nc.tensor.matmul(out=ws_ps[:], lhsT=wt_sb[:], rhs=a_sb[:],
                             start=True, stop=True)
            nc.vector.tensor_copy(out=waug[:, OUT_DIM:OUT_DIM + 2], in_=ws_ps[:])

            edges = [0, 1024, 3072]
            while edges[-1] < NPP:
                edges.append(min(edges[-1] + 3072, NPP))
            for g0, g1 in zip(edges[:-1], edges[1:]):
                gw = g1 - g0
                ht = hp.tile([128, 3072], bf16, tag="ht")
                nc.sync.dma_start(out=ht[:, :gw], in_=d_hT[:, g0:g1])
                wh_sb = wo.tile([OUT_DIM, 3072], bf16, tag="wh")
                s_sb = wo.tile([2, 3072], f32, tag="s")
                for c0 in range(0, gw, NW):
                    w = min(c0 + NW, gw) - c0
                    wh_ps = psp.tile([OUT_DIM + 2, NW], f32, space="PSUM")
                    nc.tensor.matmul(out=wh_ps[:, :w], lhsT=waug[:],
                                     rhs=ht[:, c0:c0 + w], start=True, stop=True)
                    nc.vector.tensor_copy(out=wh_sb[:32, c0:c0 + w],
                                          in_=wh_ps[:32, :w])
                    nc.scalar.activation(out=wh_sb[32:64, c0:c0 + w],
                                         in_=wh_ps[32:64, :w], func=act.Copy)
                    nc.vector.tensor_copy(out=s_sb[:, c0:c0 + w],
                                          in_=wh_ps[64:66, :w])
                nc.sync.dma_start(out=d_whT[:, g0:g1], in_=wh_sb[:, :gw])
                nc.sync.dma_start(out=d_sT[:, g0:g1], in_=s_sb[:, :gw])
    nc.compile()
    return nc


def _chunks_of(Tj):
    chunks = []
    j0 = 0
    acc = 0
    for j in range(NBPC):
        acc += int(Tj[j])
        tgt = CHUNK_TILES // 4 if len(chunks) < 2 else CHUNK_TILES
        if acc >= tgt or j == NBPC - 1:
            chunks.append((j0, j + 1))
            j0 = j + 1
            acc = 0
    return chunks


def _build_pass2(Tj, Ttot):
    from concourse import bacc, mybir
    import concourse.tile as tile

    f32 = mybir.dt.float32
    bf16 = mybir.dt.bfloat16
    f16 = mybir.dt.float16
    alu = mybir.AluOpType
    act = mybir.ActivationFunctionType

    base = np.zeros(NBPC + 1, np.int64)
    base[1:] = np.cumsum(Tj)
    assert base[-1] == Ttot

    nc = bacc.Bacc("TRN2", target_bir_lowering=False, debug=False)
    d_msg = nc.dram_tensor("msg", [128, Ttot * OUT_DIM], bf16, kind="ExternalInput")
    d_q = nc.dram_tensor("q", [128, 2 * Ttot], f16, kind="ExternalInput")
    d_I = nc.dram_tensor("I", [128, 128], bf16, kind="ExternalInput")
    d_out = nc.dram_tensor("out", [128, NBPC * OUT_DIM], bf16, kind="ExternalOutput")

    chunks = _chunks_of(Tj)
    max_cb = max(j1 - j0 for j0, j1 in chunks)

    with tile.TileContext(nc) as tc:
        with tc.tile_pool(name="c2", bufs=1) as cp, \
             tc.tile_pool(name="mg", bufs=4) as mp, \
             tc.tile_pool(name="ob", bufs=3) as op, \
             tc.tile_pool(name="pp", bufs=8, space="PSUM") as pp:

            I_sb = cp.tile([128, 128], bf16)
            nc.sync.dma_start(out=I_sb[:], in_=d_I[:])

            q_sb = cp.tile([128, 2 * Ttot], f16)
            ex_sb = cp.tile([128, Ttot], bf16)
            ex2_sb = cp.tile([128, 2 * Ttot], bf16)
            den_sb = cp.tile([128, NBPC], f32)
            dinv_sb = cp.tile([128, NBPC], f32)

            def q_chain(lo, hi):
                for k in range(2):
                    nc.sync.dma_start(
                        out=q_sb[:, k * Ttot + lo:k * Ttot + hi],
                        in_=d_q[:, k * Ttot + lo:k * Ttot + hi])
                qs = q_sb[:, lo:hi]
                qd = q_sb[:, Ttot + lo:Ttot + hi]
                nc.gpsimd.tensor_tensor(out=qs, in0=qs, in1=qd, op=alu.add)
                nc.vector.scalar_tensor_tensor(
                    out=qs, in0=qs, scalar=float(NEG_SLOPE), in1=qs,
                    op0=alu.mult, op1=alu.max)
                nc.scalar.activation(out=ex_sb[:, lo:hi], in_=qs, func=act.Exp)
                eout = ex2_sb[:, 2 * lo:2 * hi].rearrange(
                    "p (t two) -> p t two", two=2)
                ein = ex_sb[:, lo:hi].rearrange("p (t o) -> p t o", o=1)
                nc.gpsimd.tensor_copy(
                    out=eout, in_=ein.to_broadcast([128, hi - lo, 2]))

            cut = int(base[chunks[0][1]])
            q_chain(0, cut)

            for ci, (j0, j1) in enumerate(chunks):
                if ci == 1:
                    q_chain(cut, Ttot)
                t0, t1 = int(base[j0]), int(base[j1])
                CT = t1 - t0
                X = mp.tile([128, CT * OUT_DIM], bf16, tag="X")
                nc.sync.dma_start(out=X[:],
                                  in_=d_msg[:, t0 * OUT_DIM:t1 * OUT_DIM])

                tg = t0 + (CT * DVE_NUM) // DVE_DEN
                if tg > t0:
                    sl = slice(0, (tg - t0) * OUT_DIM)
                    in1 = ex2_sb[:, 2 * t0:2 * tg].rearrange(
                        "p (t r two) -> p t r two", r=1, two=2)
                    nc.vector.tensor_tensor(
                        out=X[:, sl], in0=X[:, sl],
                        in1=in1.to_broadcast([128, tg - t0, OUT_DIM // 2, 2]),
                        op=alu.mult)
                if t1 > tg:
                    sl = slice((tg - t0) * OUT_DIM, CT * OUT_DIM)
                    in1 = ex_sb[:, tg:t1].rearrange("p (t o) -> p t o", o=1)
                    nc.gpsimd.tensor_tensor(
                        out=X[:, sl], in0=X[:, sl],
                        in1=in1.to_broadcast([128, t1 - tg, OUT_DIM]),
                        op=alu.mult)

                for j in range(j0, j1):
                    nc.vector.tensor_reduce(
                        out=den_sb[:, j:j + 1],
                        in_=ex_sb[:, int(base[j]):int(base[j + 1])],
                        op=alu.add, axis=mybir.AxisListType.X)
                dn = dinv_sb[:, j0:j1]
                nc.vector.tensor_scalar(out=dn, in0=den_sb[:, j0:j1],
                                        scalar1=1e-10, scalar2=None, op0=alu.add)
                nc.vector.reciprocal(out=dn, in_=dn)

                out_sb = op.tile([128, max_cb * OUT_DIM], bf16, tag="o")
                for j in range(j0, j1):
                    tj = int(Tj[j])
                    bj = int(base[j])
                    ps = pp.tile([128, OUT_DIM], f32, space="PSUM", tag="ps")
                    for t in range(tj):
                        rel = bj + t - t0
                        nc.tensor.matmul(
                            out=ps[:], lhsT=I_sb[:],
                            rhs=X[:, rel * OUT_DIM:(rel + 1) * OUT_DIM],
                            start=(t == 0), stop=(t == tj - 1))
                    jr = j - j0
                    nc.scalar.activation(
                        out=out_sb[:, jr * OUT_DIM:(jr + 1) * OUT_DIM],
                        in_=ps[:], func=act.Copy, scale=dinv_sb[:, j:j + 1])
                nc.sync.dma_start(
                    out=d_out[:, j0 * OUT_DIM:j1 * OUT_DIM],
                    in_=out_sb[:, 0:(j1 - j0) * OUT_DIM])
    nc.compile()
    return nc


def _run_spmd(nc, in_maps, trace=False):
    from concourse import bass_utils
    res = bass_utils.run_bass_kernel_spmd(
        nc, in_maps, core_ids=list(range(CORES)), trace=trace)
    return res


def kernel(h, row, col, W, a):
    trace = bool(os.environ.get("GAT_TRACE"))
    if trace:
        try:
            import ntff_shim
            ntff_shim.install()
        except Exception:
            trace = False

    bf = ml_dtypes.bfloat16
    h = np.asarray(h, dtype=np.float32)
    W = np.asarray(W, dtype=np.float32)
    a = np.asarray(a, dtype=np.float32).reshape(2 * OUT_DIM)
    row = np.asarray(row).astype(np.int64)
    col = np.asarray(col).astype(np.int64)

    # ---- pass 1 ----
    nc1 = _build_pass1()
    W_bf = W.astype(bf)
    WT_bf = np.ascontiguousarray(W.T).astype(bf)
    a2_bf = np.ascontiguousarray(np.stack([a[:OUT_DIM], a[OUT_DIM:]], axis=1)).astype(bf)
    in_maps1 = []
    NPC = N_NODES // CORES
    for c in range(CORES):
        hpad = np.zeros((NPP, IN_DIM), np.float32)
        hpad[:NPC] = h[c * NPC:(c + 1) * NPC]
        in_maps1.append({"hT": np.ascontiguousarray(hpad.T).astype(bf),
                         "Wm": W_bf, "WT": WT_bf, "a2": a2_bf})
    res1 = _run_spmd(nc1, in_maps1, trace=trace)
    if trace:
        LAST_STATS["pass1_ns"] = res1.exec_time_ns

    Wh_bf = np.empty((N_NODES, OUT_DIM), dtype=bf)
    s_src = np.empty(N_NODES, np.float32)
    s_dst = np.empty(N_NODES, np.float32)
    for c in range(CORES):
        whT = res1.results[c]["whT"]
        sT = res1.results[c]["sT"]
        Wh_bf[c * NPC:(c + 1) * NPC] = whT[:, :NPC].T
        s_src[c * NPC:(c + 1) * NPC] = sT[0, :NPC]
        s_dst[c * NPC:(c + 1) * NPC] = sT[1, :NPC]

    # ---- host: structure + streams ----
    deg = np.bincount(row, minlength=N_NODES)
    perm = np.argsort(-deg, kind="stable")
    slot_of = np.empty(N_NODES, np.int64)
    slot_of[perm] = np.arange(N_NODES)
    blk = slot_of // 128
    p_of = slot_of % 128
    core_of_node = blk % CORES
    j_of_node = blk // CORES

    deg_sorted = deg[perm]
    starts_idx = np.minimum(np.arange(NBPC) * 8 * 128, N_NODES - 1)
    Tj = np.maximum(1, deg_sorted[starts_idx])
    base = np.zeros(NBPC + 1, np.int64)
    base[1:] = np.cumsum(Tj)
    Ttot = int(base[-1])

    order = np.argsort(row, kind="stable")
    cnt = np.bincount(row, minlength=N_NODES)
    starts = np.zeros(N_NODES, np.int64)
    starts[1:] = np.cumsum(cnt)[:-1]
    t_rank = np.empty(N_EDGES, np.int64)
    t_rank[order] = np.arange(N_EDGES) - np.repeat(starts, cnt)

    e_core = core_of_node[row]
    e_p = p_of[row]
    e_col = base[j_of_node[row]] + t_rank

    Wh_u16 = Wh_bf.view(np.uint16)
    msg = np.zeros((CORES, 128, Ttot, OUT_DIM), np.uint16)
    msg[e_core, e_p, e_col] = Wh_u16[col]
    q = np.full((CORES, 128, 2, Ttot), PAD_Q, np.float16)
    q[e_core, e_p, 0, e_col] = s_src[row].astype(np.float16)
    q[e_core, e_p, 1, e_col] = s_dst[col].astype(np.float16)

    # ---- pass 2 ----
    nc2 = _build_pass2(Tj, Ttot)
    I_bf = np.eye(128, dtype=bf)
    in_maps2 = [{"msg": msg[c].reshape(128, Ttot * OUT_DIM).view(bf),
                 "q": q[c].reshape(128, 2 * Ttot),
                 "I": I_bf}
                for c in range(CORES)]
    res2 = _run_spmd(nc2, in_maps2, trace=trace)
    if trace:
        LAST_STATS["pass2_ns"] = res2.exec_time_ns
        LAST_STATS["total_ns"] = (res1.exec_time_ns or 0) + (res2.exec_time_ns or 0)
        LAST_STATS["res1"] = res1
        LAST_STATS["res2"] = res2

    # ---- unpermute ----
    out = np.empty((N_NODES, OUT_DIM), np.float32)
    dev = np.stack([res2.results[c]["out"].reshape(128, NBPC, OUT_DIM)
                    .astype(np.float32) for c in range(CORES)])
    out[np.arange(N_NODES)] = dev[core_of_node, p_of, j_of_node]
    return out


# revision 4
# speedup vs baseline: 1.0421x; 1.0421x over previous
"""GAT influence layer on 8 Trainium2 NeuronCores (Bass/Tile) — v2.3.

Strategy (node-permuted blocks, identity-stationary segment-sum):
  Pass 1 (device): Wh = h @ W (bf16 in, f32 psum), s_src/s_dst via an
      augmented weight matrix, node-sharded across cores.  Outputs Wh
      bf16 + s f32.
  Host: degree-sort nodes; 128-node blocks; block b -> (core b%8,
      row j=b//8); node at partition p = slot%128.  Edge (row,col) ->
      (core, p, tile column base[j]+rank).  Streams: msg = Wh[col] bf16,
      qs/qd fp16.  Pure gather/permutation.
  Pass 2 (device): exp(leakyrelu(qs+qd)) -> bf16 (+ pair-duplicated
      copy so the scaling tensor_tensor gets an innermost step-1 AP);
      X = msg * exp (DVE/GPSIMD split); segment sum via PSUM-accumulated
      [128,64] matmuls with a CONSTANT identity stationary; per-block
      denominator = reduce_sum(exp) scheduled per chunk; epilogue
      Copy*(1/den) on ScalarE, bf16 out.  The reference's global
      max-subtract cancels in the softmax ratio.
  Host: unpermute rows.
"""

import os
import numpy as np
import ml_dtypes

N_NODES = 100000
N_EDGES = 1600000
IN_DIM = 128
OUT_DIM = 64
NEG_SLOPE = 0.2
CORES = 8
NBPC = 98                     # 128-node blocks per core
NPP = NBPC * 128              # padded nodes per core (12544)
PAD_Q = -30000.0
CHUNK_TILES = 176             # target tiles per pass-2 pipeline chunk
DVE_NUM = 6                   # DVE takes DVE_NUM/DVE_DEN of scaling work
DVE_DEN = 7

LAST_STATS = {}


def _build_pass1():
    from concourse import bacc, mybir
    import concourse.tile as tile

    f32 = mybir.dt.float32
    bf16 = mybir.dt.bfloat16
    act = mybir.ActivationFunctionType
    nc = bacc.Bacc("TRN2", target_bir_lowering=False, debug=False)
    d_hT = nc.dram_tensor("hT", [128, NPP], bf16, kind="ExternalInput")
    d_W = nc.dram_tensor("Wm", [IN_DIM, OUT_DIM], bf16, kind="ExternalInput")
    d_WT = nc.dram_tensor("WT", [OUT_DIM, IN_DIM], bf16, kind="ExternalInput")
    d_a2 = nc.dram_tensor("a2", [OUT_DIM, 2], bf16, kind="ExternalInput")
    d_whT = nc.dram_tensor("whT", [OUT_DIM, NPP], bf16, kind="ExternalOutput")
    d_sT = nc.dram_tensor("sT", [2, NPP], f32, kind="ExternalOutput")

    NW = 512
    with tile.TileContext(nc) as tc:
        with tc.tile_pool(name="c1", bufs=1) as cp, \
             tc.tile_pool(name="ht1", bufs=6) as hp, \
             tc.tile_pool(name="wo1", bufs=4) as wo, \
             tc.tile_pool(name="psw", bufs=1, space="PSUM") as psw, \
             tc.tile_pool(name="ps1", bufs=6, space="PSUM") as psp:
            w_sb = cp.tile([IN_DIM, OUT_DIM], bf16)
            nc.sync.dma_start(out=w_sb[:], in_=d_W[:])
            wt_sb = cp.tile([OUT_DIM, IN_DIM], bf16)
            nc.sync.dma_start(out=wt_sb[:], in_=d_WT[:])
            a_sb = cp.tile([OUT_DIM, 2], bf16)
            nc.sync.dma_start(out=a_sb[:], in_=d_a2[:])

            waug = cp.tile([IN_DIM, OUT_DIM + 2], bf16)
            nc.vector.tensor_copy(out=waug[:, 0:OUT_DIM], in_=w_sb[:])
            ws_ps = psw.tile([IN_DIM, 2], f32, space="PSUM")
            nc.tensor.matmul(out=ws_ps[:], lhsT=wt_sb[:], rhs=a_sb[:],
                             start=True, stop=True)
            nc.vector.tensor_copy(out=waug[:, OUT_DIM:OUT_DIM + 2], in_=ws_ps[:])

            edges = [0, 1024, 3072]
            while edges[-1] < NPP:
                edges.append(min(edges[-1] + 3072, NPP))
            for g0, g1 in zip(edges[:-1], edges[1:]):
                gw = g1 - g0
                ht = hp.tile([128, 3072], bf16, tag="ht")
                nc.sync.dma_start(out=ht[:, :gw], in_=d_hT[:, g0:g1])
                wh_sb = wo.tile([OUT_DIM, 3072], bf16, tag="wh")
                s_sb = wo.tile([2, 3072], f32, tag="s")
                for c0 in range(0, gw, NW):
                    w = min(c0 + NW, gw) - c0
                    wh_ps = psp.tile([OUT_DIM + 2, NW], f32, space="PSUM")
                    nc.tensor.matmul(out=wh_ps[:, :w], lhsT=waug[:],
                                     rhs=ht[:, c0:c0 + w], start=True, stop=True)
                    nc.vector.tensor_copy(out=wh_sb[:32, c0:c0 + w],
                                          in_=wh_ps[:32, :w])
                    nc.scalar.activation(out=wh_sb[32:64, c0:c0 + w],
                                         in_=wh_ps[32:64, :w], func=act.Copy)
                    nc.vector.tensor_copy(out=s_sb[:, c0:c0 + w],
                                          in_=wh_ps[64:66, :w])
                nc.sync.dma_start(out=d_whT[:, g0:g1], in_=wh_sb[:, :gw])
                nc.sync.dma_start(out=d_sT[:, g0:g1], in_=s_sb[:, :gw])
    nc.compile()
    return nc


def _chunks_of(Tj):
    chunks = []
    j0 = 0
    acc = 0
    for j in range(NBPC):
        acc += int(Tj[j])
        tgt = CHUNK_TILES // 4 if len(chunks) < 2 else CHUNK_TILES
        if acc >= tgt or j == NBPC - 1:
            chunks.append((j0, j + 1))
            j0 = j + 1
            acc = 0
    return chunks


def _build_pass2(Tj, Ttot):
    from concourse import bacc, mybir
    import concourse.tile as tile

    f32 = mybir.dt.float32
    bf16 = mybir.dt.bfloat16
    f16 = mybir.dt.float16
    alu = mybir.AluOpType
    act = mybir.ActivationFunctionType

    base = np.zeros(NBPC + 1, np.int64)
    base[1:] = np.cumsum(Tj)
    assert base[-1] == Ttot

    nc = bacc.Bacc("TRN2", target_bir_lowering=False, debug=False)
    d_msg = nc.dram_tensor("msg", [128, Ttot * OUT_DIM], bf16, kind="ExternalInput")
    d_q = nc.dram_tensor("q", [128, 2 * Ttot], f16, kind="ExternalInput")
    d_I = nc.dram_tensor("I", [128, 128], bf16, kind="ExternalInput")
    d_out = nc.dram_tensor("out", [128, NBPC * OUT_DIM], bf16, kind="ExternalOutput")

    chunks = _chunks_of(Tj)
    max_cb = max(j1 - j0 for j0, j1 in chunks)

    with tile.TileContext(nc) as tc:
        with tc.tile_pool(name="c2", bufs=1) as cp, \
             tc.tile_pool(name="mg", bufs=4) as mp, \
             tc.tile_pool(name="ob", bufs=3) as op, \
             tc.tile_pool(name="pp", bufs=8, space="PSUM") as pp:

            I_sb = cp.tile([128, 128], bf16)
            nc.sync.dma_start(out=I_sb[:], in_=d_I[:])

            q_sb = cp.tile([128, 2 * Ttot], f16)
            ex_sb = cp.tile([128, Ttot], bf16)
            ex2_sb = cp.tile([128, 2 * Ttot], bf16)
            den_sb = cp.tile([128, NBPC], f32)
            dinv_sb = cp.tile([128, NBPC], f32)

            def q_chain(lo, hi):
                for k in range(2):
                    nc.sync.dma_start(
                        out=q_sb[:, k * Ttot + lo:k * Ttot + hi],
                        in_=d_q[:, k * Ttot + lo:k * Ttot + hi])
                qs = q_sb[:, lo:hi]
                qd = q_sb[:, Ttot + lo:Ttot + hi]
                nc.gpsimd.tensor_tensor(out=qs, in0=qs, in1=qd, op=alu.add)
                nc.vector.scalar_tensor_tensor(
                    out=qs, in0=qs, scalar=float(NEG_SLOPE), in1=qs,
                    op0=alu.mult, op1=alu.max)
                nc.scalar.activation(out=ex_sb[:, lo:hi], in_=qs, func=act.Exp)
                eout = ex2_sb[:, 2 * lo:2 * hi].rearrange(
                    "p (t two) -> p t two", two=2)
                ein = ex_sb[:, lo:hi].rearrange("p (t o) -> p t o", o=1)
                nc.gpsimd.tensor_copy(
                    out=eout, in_=ein.to_broadcast([128, hi - lo, 2]))

            def q_slice(ci):
                if ci < len(chunks):
                    q_chain(int(base[chunks[ci][0]]), int(base[chunks[ci][1]]))

            q_slice(0)
            q_slice(1)

            for ci, (j0, j1) in enumerate(chunks):
                t0, t1 = int(base[j0]), int(base[j1])
                CT = t1 - t0
                X = mp.tile([128, CT * OUT_DIM], bf16, tag="X")
                nc.sync.dma_start(out=X[:],
                                  in_=d_msg[:, t0 * OUT_DIM:t1 * OUT_DIM])

                for j in range(j0, j1):
                    nc.vector.tensor_reduce(
                        out=den_sb[:, j:j + 1],
                        in_=ex_sb[:, int(base[j]):int(base[j + 1])],
                        op=alu.add, axis=mybir.AxisListType.X)
                dn = dinv_sb[:, j0:j1]
                nc.vector.tensor_scalar(out=dn, in0=den_sb[:, j0:j1],
                                        scalar1=1e-10, scalar2=None, op0=alu.add)
                nc.vector.reciprocal(out=dn, in_=dn)

                tg = t0 + (CT * DVE_NUM) // DVE_DEN
                if tg > t0:
                    sl = slice(0, (tg - t0) * OUT_DIM)
                    in1 = ex2_sb[:, 2 * t0:2 * tg].rearrange(
                        "p (t r two) -> p t r two", r=1, two=2)
                    nc.vector.tensor_tensor(
                        out=X[:, sl], in0=X[:, sl],
                        in1=in1.to_broadcast([128, tg - t0, OUT_DIM // 2, 2]),
                        op=alu.mult)
                if t1 > tg:
                    sl = slice((tg - t0) * OUT_DIM, CT * OUT_DIM)
                    in1 = ex_sb[:, tg:t1].rearrange("p (t o) -> p t o", o=1)
                    nc.gpsimd.tensor_tensor(
                        out=X[:, sl], in0=X[:, sl],
                        in1=in1.to_broadcast([128, t1 - tg, OUT_DIM]),
                        op=alu.mult)

                q_slice(ci + 2)

                out_sb = op.tile([128, max_cb * OUT_DIM], bf16, tag="o")
                for j in range(j0, j1):
                    tj = int(Tj[j])
                    bj = int(base[j])
                    ps = pp.tile([128, OUT_DIM], f32, space="PSUM", tag="ps")
                    for t in range(tj):
                        rel = bj + t - t0
                        nc.tensor.matmul(
                            out=ps[:], lhsT=I_sb[:],
                            rhs=X[:, rel * OUT_DIM:(rel + 1) * OUT_DIM],
                            start=(t == 0), stop=(t == tj - 1))
                    jr = j - j0
                    nc.scalar.activation(
                        out=out_sb[:, jr * OUT_DIM:(jr + 1) * OUT_DIM],
                        in_=ps[:], func=act.Copy, scale=dinv_sb[:, j:j + 1])
                nc.sync.dma_start(
                    out=d_out[:, j0 * OUT_DIM:j1 * OUT_DIM],
                    in_=out_sb[:, 0:(j1 - j0) * OUT_DIM])
    nc.compile()
    return nc


def _run_spmd(nc, in_maps, trace=False):
    from concourse import bass_utils
    res = bass_utils.run_bass_kernel_spmd(
        nc, in_maps, core_ids=list(range(CORES)), trace=trace)
    return res


def kernel(h, row, col, W, a):
    trace = bool(os.environ.get("GAT_TRACE"))
    if trace:
        try:
            import ntff_shim
            ntff_shim.install()
        except Exception:
            trace = False

    bf = ml_dtypes.bfloat16
    h = np.asarray(h, dtype=np.float32)
    W = np.asarray(W, dtype=np.float32)
    a = np.asarray(a, dtype=np.float32).reshape(2 * OUT_DIM)
    row = np.asarray(row).astype(np.int64)
    col = np.asarray(col).astype(np.int64)

    # ---- pass 1 ----
    nc1 = _build_pass1()
    W_bf = W.astype(bf)
    WT_bf = np.ascontiguousarray(W.T).astype(bf)
    a2_bf = np.ascontiguousarray(np.stack([a[:OUT_DIM], a[OUT_DIM:]], axis=1)).astype(bf)
    in_maps1 = []
    NPC = N_NODES // CORES
    for c in range(CORES):
        hpad = np.zeros((NPP, IN_DIM), np.float32)
        hpad[:NPC] = h[c * NPC:(c + 1) * NPC]
        in_maps1.append({"hT": np.ascontiguousarray(hpad.T).astype(bf),
                         "Wm": W_bf, "WT": WT_bf, "a2": a2_bf})
    res1 = _run_spmd(nc1, in_maps1, trace=trace)
    if trace:
        LAST_STATS["pass1_ns"] = res1.exec_time_ns

    Wh_bf = np.empty((N_NODES, OUT_DIM), dtype=bf)
    s_src = np.empty(N_NODES, np.float32)
    s_dst = np.empty(N_NODES, np.float32)
    for c in range(CORES):
        whT = res1.results[c]["whT"]
        sT = res1.results[c]["sT"]
        Wh_bf[c * NPC:(c + 1) * NPC] = whT[:, :NPC].T
        s_src[c * NPC:(c + 1) * NPC] = sT[0, :NPC]
        s_dst[c * NPC:(c + 1) * NPC] = sT[1, :NPC]

    # ---- host: structure + streams ----
    deg = np.bincount(row, minlength=N_NODES)
    perm = np.argsort(-deg, kind="stable")
    slot_of = np.empty(N_NODES, np.int64)
    slot_of[perm] = np.arange(N_NODES)
    blk = slot_of // 128
    p_of = slot_of % 128
    core_of_node = blk % CORES
    j_of_node = blk // CORES

    deg_sorted = deg[perm]
    starts_idx = np.minimum(np.arange(NBPC) * 8 * 128, N_NODES - 1)
    Tj = np.maximum(1, deg_sorted[starts_idx])
    base = np.zeros(NBPC + 1, np.int64)
    base[1:] = np.cumsum(Tj)
    Ttot = int(base[-1])

    order = np.argsort(row, kind="stable")
    cnt = np.bincount(row, minlength=N_NODES)
    starts = np.zeros(N_NODES, np.int64)
    starts[1:] = np.cumsum(cnt)[:-1]
    t_rank = np.empty(N_EDGES, np.int64)
    t_rank[order] = np.arange(N_EDGES) - np.repeat(starts, cnt)

    e_core = core_of_node[row]
    e_p = p_of[row]
    e_col = base[j_of_node[row]] + t_rank

    Wh_u16 = Wh_bf.view(np.uint16)
    msg = np.zeros((CORES, 128, Ttot, OUT_DIM), np.uint16)
    msg[e_core, e_p, e_col] = Wh_u16[col]
    q = np.full((CORES, 128, 2, Ttot), PAD_Q, np.float16)
    q[e_core, e_p, 0, e_col] = s_src[row].astype(np.float16)
    q[e_core, e_p, 1, e_col] = s_dst[col].astype(np.float16)

    # ---- pass 2 ----
    nc2 = _build_pass2(Tj, Ttot)
    I_bf = np.eye(128, dtype=bf)
    in_maps2 = [{"msg": msg[c].reshape(128, Ttot * OUT_DIM).view(bf),
                 "q": q[c].reshape(128, 2 * Ttot),
                 "I": I_bf}
                for c in range(CORES)]
    res2 = _run_spmd(nc2, in_maps2, trace=trace)
    if trace:
        LAST_STATS["pass2_ns"] = res2.exec_time_ns
        LAST_STATS["total_ns"] = (res1.exec_time_ns or 0) + (res2.exec_time_ns or 0)
        LAST_STATS["res1"] = res1
        LAST_STATS["res2"] = res2

    # ---- unpermute ----
    out = np.empty((N_NODES, OUT_DIM), np.float32)
    dev = np.stack([res2.results[c]["out"].reshape(128, NBPC, OUT_DIM)
                    .astype(np.float32) for c in range(CORES)])
    out[np.arange(N_NODES)] = dev[core_of_node, p_of, j_of_node]
    return out
